# revision 17
# baseline (speedup 1.0000x reference)
"""Trainium2 Bass kernel for nn_CausalSelfAttention_newsim_weight.

Reference computation (B=2, T=2048, C=1024, H=16, d=64):
    qkv = x @ Wv.T + bv                  # q = k = v = qkv
    w   = weight[:, :, 0]
    v   = qkv * w[:, :, None] * v_scale
    att = (q @ k.T) * w[s] * att_scale / sqrt(d)   (causal, softmax over s)
    y   = att @ v ;  out = y @ Wp.T + bp

Sharding: 8 cores = 2 batches x 4 head-groups (4 heads each). No
collectives: each core emits a partial projection out_part = y_hg @ Wp[:, cols].T
and the host sums the 4 partials per batch and adds bp.

Per-core dataflow (all matmuls bf16, fp32 PSUM accumulation):
  1. qkvT[o, t] = WvT_hg.T @ xT  (+bv via ACT per-partition bias)
  2. vw[s, d]   = PE-transpose of qkvT blocks, scaled by w[s]*v_scale
     (per-partition scale after transpose); augmented with a ones column
     so the attention V-matmul also produces softmax denominators.
  3. attT[s, t] = kT.T @ qT per head; exp via ACT with per-partition
     scale w[s]*att_scale/8 (s is the partition axis in attT, so the
     per-key weighting is a native ACT scale vector).  Causal masking via
     a host-supplied 0/1 mask on the block-diagonal chunks.
  4. yT_unnorm[d, t] (+denominator row 64) accumulates over s-blocks in
     PSUM; normalization by 1/denominator broadcast via a tiny selector
     matmul on the PE.
  5. out_part[t, oo] = ynT.T @ WpT (partial projection, fp32 out).
"""

import sys

sys.path.insert(0, "/opt/trn_rl_repo")

import numpy as np
import ml_dtypes

import concourse.bass as bass
import concourse.mybir as mybir
import concourse.tile as tile
from concourse import bacc
from concourse.bass_utils import run_bass_kernel_spmd
from concourse.masks import make_identity

BF16 = mybir.dt.bfloat16
F32 = mybir.dt.float32
AFT = mybir.ActivationFunctionType

B, T, C, H, D = 2, 2048, 1024, 16, 64
P = 128
HG = 4           # heads per core
OC = HG * D      # 256 qkv columns per core
NSB = T // P     # 16 s-blocks of 128
TCH = 512        # t-chunk size
NTC = T // TCH   # 4 t-chunks
KB = C // P      # 8 contraction blocks for qkv


def _build_body(tc, d):
    nc = tc.nc
    from contextlib import ExitStack

    with ExitStack() as ctx:
        const = ctx.enter_context(tc.tile_pool(name="const", bufs=1))

        # ---- resident inputs ----
        xT = [const.tile([P, T], BF16, tag=f"xT{i}", name=f"xT{i}") for i in range(KB)]
        for i in range(KB):
            nc.sync.dma_start(xT[i][:], d["xT"][i * P:(i + 1) * P, :])
        WvT = [const.tile([P, OC], BF16, tag=f"WvT{i}", name=f"WvT{i}") for i in range(KB)]
        for i in range(KB):
            nc.sync.dma_start(WvT[i][:], d["WvT"][i * P:(i + 1) * P, :])
        WpT = [const.tile([P, C], BF16, tag=f"WpT{i}", name=f"WpT{i}") for i in range(2)]
        for i in range(2):
            nc.sync.dma_start(WpT[i][:], d["WpT"][i * P:(i + 1) * P, :])
        bv = const.tile([P, 2], F32, tag="bv", name="bv")
        nc.sync.dma_start(bv[:], d["bv"][:, :])
        wq = const.tile([P, NSB], F32, tag="wq", name="wq")
        nc.sync.dma_start(wq[:], d["wq"][:, :])
        wv = const.tile([P, NSB], F32, tag="wv", name="wv")
        nc.sync.dma_start(wv[:], d["wv"][:, :])
        maskt = const.tile([P, 4, 2 * TCH], BF16, tag="maskd", name="maskd")
        nc.sync.dma_start(maskt[:], d["maskd"][:, :, :])

        ident = const.tile([P, P], BF16, tag="ident", name="ident")
        make_identity(nc, ident[:])
        # ones row for broadcasting a reciprocal row to 64 partitions
        ones1 = const.tile([1, D], F32, tag="ones1", name="ones1")
        nc.gpsimd.memset(ones1[:], 1.0)

        qkvT = [const.tile([P, T], BF16, tag=f"qkvT{ob}", name=f"qkvT{ob}") for ob in range(2)]
        ynT = [const.tile([P, T], BF16, tag=f"ynT{ob}", name=f"ynT{ob}") for ob in range(2)]
        vw = [[const.tile([P, D + 1], BF16, tag=f"vw{h}_{sb}", name=f"vw{h}_{sb}")
               for sb in range(NSB)] for h in range(HG)]

        # ---- single open-pool schedule ----
        # PSUM banks: qkv 1 + (att|tr shared) 4 + y 2 + proj 1 = 8
        with tc.tile_pool(name="ps_qkv", bufs=1, space="PSUM") as ps_qkv, \
             tc.tile_pool(name="ps_tr", bufs=1, space="PSUM") as ps_tr, \
             tc.tile_pool(name="ps_att", bufs=2, space="PSUM") as ps_att, \
             tc.tile_pool(name="ps_y", bufs=1, space="PSUM") as ps_y, \
             tc.tile_pool(name="op", bufs=3) as op, \
             tc.tile_pool(name="pp", bufs=3) as pp, \
             tc.tile_pool(name="rcp", bufs=2) as rcp:
            # qkvT, t-chunk-major so attention chunk 0 unblocks early
            for tcn in range(NTC):
                for ob in range(2):
                    ps = ps_qkv.tile([P, TCH], F32, tag="qk", name="qk")
                    for kb in range(KB):
                        nc.tensor.matmul(
                            ps[:],
                            WvT[kb][:, ob * P:(ob + 1) * P],
                            xT[kb][:, tcn * TCH:(tcn + 1) * TCH],
                            start=(kb == 0), stop=(kb == KB - 1),
                        )
                    nc.vector.tensor_scalar_add(
                        qkvT[ob][:, tcn * TCH:(tcn + 1) * TCH], ps[:],
                        bv[:, ob:ob + 1],
                    )
                # vw tiles for this chunk's 4 s-blocks (psum slots shared
                # with the attention pool)
                for hp in range(2):
                    for sbl in range(4):
                        sb = 4 * tcn + sbl
                        pst = ps_tr.tile([P, P], BF16, tag="tr", name="tr")
                        nc.tensor.transpose(
                            pst[:], qkvT[hp][:, sb * P:(sb + 1) * P], ident[:])
                        for hh in range(2):
                            h = 2 * hp + hh
                            nc.vector.tensor_scalar_mul(
                                vw[h][sb][:, 0:D], pst[:, hh * D:(hh + 1) * D],
                                wv[:, sb:sb + 1])
                            nc.gpsimd.memset(vw[h][sb][:, D:D + 1], 1.0)
            # attention + projection per t-chunk
            for tcn in range(NTC):
                tsl = slice(tcn * TCH, (tcn + 1) * TCH)
                nsb = 4 * (tcn + 1)
                for hp in range(2):
                    yp = [ps_y.tile([D + 1, TCH], F32, tag=f"yp{hh}", name=f"yp{hh}")
                          for hh in range(2)]
                    for sb in range(nsb):
                        ap_ = ps_att.tile([P, 2 * TCH], F32, tag="ap", name="ap")
                        for hh in range(2):
                            dsl = slice(hh * D, (hh + 1) * D)
                            nc.tensor.matmul(
                                ap_[:, hh * TCH:(hh + 1) * TCH],
                                qkvT[hp][dsl, sb * P:(sb + 1) * P],
                                qkvT[hp][dsl, tsl],
                                start=True, stop=True,
                            )
                        pt = pp.tile([P, 2 * TCH], BF16)
                        nc.scalar.activation(
                            pt[:], ap_[:], AFT.Exp, scale=wq[:, sb:sb + 1])
                        if sb >= 4 * tcn:
                            nc.vector.tensor_mul(
                                pt[:], pt[:], maskt[:, sb - 4 * tcn, :])
                        for hh in range(2):
                            h = 2 * hp + hh
                            nc.tensor.matmul(
                                yp[hh][:],
                                vw[h][sb][:],
                                pt[:, hh * TCH:(hh + 1) * TCH],
                                start=(sb == 0), stop=(sb == nsb - 1),
                            )
                    for hh in range(2):
                        rc = rcp.tile([1, TCH], F32, tag="rc", name="rc")
                        nc.vector.reciprocal(rc[:], yp[hh][D:D + 1, :])
                        rbs = rcp.tile([D, TCH], F32, tag="rbs", name="rbs")
                        nc.gpsimd.partition_broadcast(rbs[:], rc[:])
                        nc.vector.tensor_mul(
                            ynT[hp][hh * D:(hh + 1) * D, tsl],
                            yp[hh][0:D, :], rbs[:])
                # partial projection for this t-chunk
                for tbl in range(4):
                    tb = 4 * tcn + tbl
                    for oc2 in range(2):
                        ps = ps_qkv.tile([P, TCH], F32, tag="qk", name="pso")
                        for kb2 in range(2):
                            nc.tensor.matmul(
                                ps[:],
                                ynT[kb2][:, tb * P:(tb + 1) * P],
                                WpT[kb2][:, oc2 * TCH:(oc2 + 1) * TCH],
                                start=(kb2 == 0), stop=(kb2 == 1),
                            )
                        ot = op.tile([P, TCH], F32, tag="ot", name="ot")
                        nc.vector.tensor_copy(out=ot[:], in_=ps[:])
                        nc.sync.dma_start(
                            d["out"][tb * P:(tb + 1) * P,
                                     oc2 * TCH:(oc2 + 1) * TCH],
                            ot[:])


def build(repeat=1):
    nc = bacc.Bacc("TRN2", target_bir_lowering=False)
    d = {}
    d["xT"] = nc.dram_tensor("xT", [C, T], BF16, kind="ExternalInput")
    d["WvT"] = nc.dram_tensor("WvT", [C, OC], BF16, kind="ExternalInput")
    d["bv"] = nc.dram_tensor("bv", [P, 2], F32, kind="ExternalInput")
    d["wq"] = nc.dram_tensor("wq", [P, NSB], F32, kind="ExternalInput")
    d["wv"] = nc.dram_tensor("wv", [P, NSB], F32, kind="ExternalInput")
    d["maskd"] = nc.dram_tensor("maskd", [P, 4, 2 * TCH], BF16,
                                kind="ExternalInput")
    d["WpT"] = nc.dram_tensor("WpT", [OC, C], BF16, kind="ExternalInput")
    d["out"] = nc.dram_tensor("out", [T, C], F32, kind="ExternalOutput")
    with tile.TileContext(nc) as tc:
        for _ in range(repeat):
            _build_body(tc, d)
    nc.compile()
    return nc


def make_in_maps(x, weight, Wv, bv, Wp, v_scale, att_scale):
    bf = ml_dtypes.bfloat16
    x = np.asarray(x, np.float32)
    w = np.asarray(weight, np.float32)[:, :, 0]
    Wv = np.asarray(Wv, np.float32)
    Wp = np.asarray(Wp, np.float32)
    bv = np.asarray(bv, np.float32)
    att_s = float(np.asarray(att_scale).reshape(-1)[0])
    v_s = float(np.asarray(v_scale).reshape(-1)[0])

    WvT_full = np.ascontiguousarray(Wv.T).astype(bf)     # [C, C]: [k, o]
    WpT_full = np.ascontiguousarray(Wp.T)                # [C, C]: [o, oo]

    # causal 0/1 mask for the 4 diagonal s-blocks of each t-chunk,
    # duplicated along free for the two heads of a pair
    pidx = np.arange(P)[:, None]
    cidx = np.arange(TCH)[None, :]
    m = np.stack([(P * i + pidx) <= cidx for i in range(4)], axis=1)
    maskd = np.concatenate([m, m], axis=2).astype(bf)    # [128, 4, 1024]

    xTb = [np.ascontiguousarray(x[b].T).astype(bf) for b in range(B)]
    wqb = [(w[b] * (att_s / 8.0)).reshape(NSB, P).T.copy() for b in range(B)]
    wvb = [(w[b] * v_s).reshape(NSB, P).T.copy() for b in range(B)]

    in_maps = []
    for core in range(8):
        b, hg = divmod(core, 4)
        cols = slice(OC * hg, OC * (hg + 1))
        in_maps.append({
            "xT": xTb[b],
            "WvT": np.ascontiguousarray(WvT_full[:, cols]),
            "bv": np.ascontiguousarray(bv[cols].reshape(2, P).T),
            "wq": wqb[b],
            "wv": wvb[b],
            "maskd": maskd,
            "WpT": np.ascontiguousarray(WpT_full[cols, :]).astype(bf),
        })
    return in_maps


_NC = None


def _get_nc():
    global _NC
    if _NC is None:
        _NC = build()
    return _NC


def kernel(x, weight, Wv, bv, Wp, bp, v_scale, att_scale, state=None,
           _want_timing=False, **_unused):
    in_maps = make_in_maps(x, weight, Wv, bv, Wp, v_scale, att_scale)
    nc = _get_nc()
    res = run_bass_kernel_spmd(nc, in_maps, core_ids=list(range(8)),
                               trace=_want_timing)
    outs = [np.asarray(res.results[i]["out"], np.float32) for i in range(8)]
    bp = np.asarray(bp, np.float32)
    out = np.stack([
        outs[0] + outs[1] + outs[2] + outs[3] + bp,
        outs[4] + outs[5] + outs[6] + outs[7] + bp,
    ])
    if _want_timing:
        return out, res
    return out
